# revision 13
# baseline (speedup 1.0000x reference)
"""Trainium2 Bass kernel for nn_Attention_31997506355363 (sparse_attention).

Sharding: 8 cores = 2 batches x 4 head-groups (4 heads of 16 each).
Each core computes its batch's full-sequence double-attend for its 4 heads,
plus the partial output projection (Wout rows for its heads); host sums the
4 head-group partials per batch.

Math notes (verified vs reference):
  - mask keeps j<=i OR j>i+512  (the strip i<j<=i+512 is masked out)
  - softmax has a per-head sink logit in the denominator only
  - |sim| <= ~6.4 so softmax runs without max-subtraction: p = exp(sim),
    denom = sum_j p + exp(sink)
  - attends are computed transposed: simT[j,i] tiles -> exp -> outT
    accumulated as v.T @ p per 128-j-block (contraction always on the
    partition dim, so no attention-matrix transposes are needed, and
    attend1's output hiddensT feeds attend2 directly)

Perf structure (v2):
  - all matmul operands bf16 (fp32 PE runs at 1/4 rate; tolerance is 2e-2)
  - x transposed by XBAR DMA-transpose (2-byte dtype) straight into SBUF;
    no PE transposes, no PSUM->SBUF copies for xT
  - everything SBUF-resident between phases; weights loaded once;
    phase-1-only pools (xT, projection weights, wide PSUM accs) released
    before the attends
  - projections run stationary-major (one Ldweights per (w-slice), 4
    full-width moving matmuls) to cut PE sequencer pressure
  - masking via DVE multiplies with constant 0/1 triangular tiles + DVE
    memsets; GPSIMD only does one-time constant setup
  - softmax denominators: ones-row matmuls accumulate alongside v.T @ e;
    reciprocal broadcast back to 128 partitions via a rank-1 PE matmul
"""

import sys

for _p in ("/opt/trn_rl_repo",):
    if _p not in sys.path:
        sys.path.insert(0, _p)

import numpy as np
import concourse.bass as bass
from concourse import bacc
import concourse.mybir as mybir
from concourse.tile import TileContext
from concourse.masks import make_identity

FP32 = mybir.dt.float32
MM_DT = mybir.dt.bfloat16
F8 = mybir.dt.float8e4
F8E5 = mybir.dt.float8e5
U8 = mybir.dt.uint8
N_CORES = 8
N = 2048            # sequence length
DQ = 1024           # model dim
HEADS = 4           # heads per core
SCALE = 0.125       # 64 ** -0.5, applied in the exp activations (raw fp8 k)
# attend1 e-tiles: 6 of 16 jb produced on DVE via the Schraudolph bit-trick
# (uint8 bits == e5m2 of exp(logit*0.125 - 2), one tensor_scalar op) to
# offload the ACT engine, which is the post-fp8 bottleneck.
LOG2E = 1.4426950408889634
E5_SCALE = 0.125 * 4.0 * LOG2E
E5_BIAS = 60.0 - 8.0 * LOG2E
JB_DVE = [(jb % 2 == 0 or jb in (5, 7, 11)) for jb in range(16)]  # 11 of 16
NB = N // 128       # 16 key blocks
PASS = 1024         # attend i-pass width (2 passes)
ACT = mybir.ActivationFunctionType

DEBUG = False
REPS = 1
PROJ_ONLY = False   # timing experiment: stop after projections


def _runs_for(jb, p):
    """i-subblock runs (in 128-col units within a 1024-wide pass) that are
    not fully masked for key-block jb.  Sub-block t covers queries
    I = 8p + t; (I, jb) is fully masked iff 1 <= jb - I <= 3."""
    skip_lo = max(0, jb - 8 * p - 3)
    skip_hi = min(8, jb - 8 * p)
    if skip_lo >= skip_hi:
        return [(0, 8)], None
    runs = []
    if skip_lo > 0:
        runs.append((0, skip_lo))
    if skip_hi < 8:
        runs.append((skip_hi, 8))
    return runs, (skip_lo, skip_hi)


def _mm_runs(jb, p):
    """Non-masked col ranges (elements, within the 1024-wide pass) for
    key-block jb, split at the 512 psum-bank boundary."""
    runs, _ = _runs_for(jb, p)
    out = []
    for (t0, t1) in runs:
        c0, c1 = t0 * 128, t1 * 128
        for h0, h1 in ((0, 512), (512, 1024)):
            a, b = max(c0, h0), min(c1, h1)
            if a < b:
                out.append((a, b))
    return out


def build_kernel(nc, tc, io):
    mm = nc.tensor.matmul

    xq, xkv = io["xq"], io["xkv"]
    wq, wk1, wv1, wk2, wv2, wout, sink = (
        io["wq"], io["wk1"], io["wv1"], io["wk2"], io["wv2"], io["wout"],
        io["sink"],
    )
    out = io["out"]

    const = tc.alloc_tile_pool(name="const", bufs=1)
    stat = tc.alloc_tile_pool(name="stat", bufs=1)
    # phase-1-only pools (released before the attends)
    xt_p = tc.alloc_tile_pool(name="xt", bufs=1)
    xin = tc.alloc_tile_pool(name="xin", bufs=1)
    wpool = tc.alloc_tile_pool(name="w", bufs=1)
    ps_w = tc.alloc_tile_pool(name="ps_w", bufs=2, space="PSUM")   # 4 banks

    # ---- constants ----
    onescol = const.tile([128, 1], F8, tag="onescol", name="onescol")
    nc.vector.memset(onescol[:], 1.0)
    expb = const.tile([128, 1], FP32, tag="expb", name="expb")
    nc.vector.memset(expb[:], -2.0)
    onesrow = const.tile([1, 128], MM_DT, tag="onesrow", name="onesrow")
    nc.vector.memset(onesrow[:], 1.0)
    ones4 = const.tile([128, HEADS], MM_DT, tag="ones4", name="ones4")
    nc.vector.memset(ones4[:], 1.0)

    # 0/1 triangular masks (e layout is [j partitions, i cols]):
    # tri_le keeps jj <= ii (diagonal block), tri_gt keeps jj > ii (block I+4)
    tri_le = const.tile([128, 128], MM_DT, tag="tri_le", name="tri_le")
    nc.gpsimd.memset(tri_le[:], 1.0)
    nc.gpsimd.affine_select(
        out=tri_le[:], in_=tri_le[:], compare_op=mybir.AluOpType.is_ge,
        fill=0.0, base=0, pattern=[[1, 128]], channel_multiplier=-1)
    tri_gt = const.tile([128, 128], MM_DT, tag="tri_gt", name="tri_gt")
    nc.gpsimd.memset(tri_gt[:], 1.0)
    nc.gpsimd.affine_select(
        out=tri_gt[:], in_=tri_gt[:], compare_op=mybir.AluOpType.is_ge,
        fill=0.0, base=-1, pattern=[[-1, 128]], channel_multiplier=1)
    # fp8 copies of the masks for attend1's fp8 e tiles
    tri_le8 = const.tile([128, 128], F8, tag="tri_le8", name="tri_le8")
    nc.gpsimd.memset(tri_le8[:], 1.0)
    nc.gpsimd.affine_select(
        out=tri_le8[:], in_=tri_le8[:], compare_op=mybir.AluOpType.is_ge,
        fill=0.0, base=0, pattern=[[1, 128]], channel_multiplier=-1)
    tri_gt8 = const.tile([128, 128], F8, tag="tri_gt8", name="tri_gt8")
    nc.gpsimd.memset(tri_gt8[:], 1.0)
    nc.gpsimd.affine_select(
        out=tri_gt8[:], in_=tri_gt8[:], compare_op=mybir.AluOpType.is_ge,
        fill=0.0, base=-1, pattern=[[-1, 128]], channel_multiplier=1)

    # ---- weights (DMAs ordered around the transposes; see below) ----
    def load_w(w_dram, cols, nm, eng):
        wt = [wpool.tile([128, cols], MM_DT, tag=f"{nm}{kt}", name=f"{nm}{kt}")
              for kt in range(8)]
        for kt in range(8):
            e = eng if not isinstance(eng, tuple) else eng[kt % 2]
            e.dma_start(out=wt[kt][:], in_=w_dram[kt * 128:(kt + 1) * 128, :])
        return wt

    wq_sb = load_w(wq, 256, "wq", (nc.sync, nc.scalar))

    # ---- persistent SBUF intermediates ----
    qT_sb = [stat.tile([128, N], F8, tag=f"qT{t}", name=f"qT{t}") for t in range(2)]
    k1T_sb = [stat.tile([128, N], F8, tag=f"k1T{t}", name=f"k1T{t}") for t in range(2)]
    k2T_sb = [stat.tile([128, N], F8, tag=f"k2T{t}", name=f"k2T{t}") for t in range(4)]
    v1h_sb = [stat.tile([128, 512], F8, tag=f"v1h{t}", name=f"v1h{t}") for t in range(NB)]
    v1l_sb = [stat.tile([128, 512], F8, tag=f"v1l{t}", name=f"v1l{t}") for t in range(NB)]
    v2a_sb = [stat.tile([128, 65 * HEADS], MM_DT, tag=f"v2a{t}", name=f"v2a{t}")
              for t in range(NB)]
    o2T = [stat.tile([128, N], MM_DT, tag=f"o2T{t}", name=f"o2T{t}") for t in range(2)]

    # =====================================================================
    # Phase 1: DMA-transpose x into SBUF, then stationary-major projections.
    # =====================================================================
    xqT = [xt_p.tile([128, N], MM_DT, tag=f"xqT{kt}", name=f"xqT{kt}")
           for kt in range(8)]
    xkvT = [xt_p.tile([128, N], MM_DT, tag=f"xkvT{kt}", name=f"xkvT{kt}")
            for kt in range(8)]

    def load_xT_half(hf):
        """DMA host-pre-transposed x directly into the xT tiles (no PE
        transposes, no PSUM->SBUF copies)."""
        c0, c1 = hf * 1024, (hf + 1) * 1024
        for kt in range(8):
            eng = nc.sync if kt % 2 == 0 else nc.scalar
            eng.dma_start(out=xqT[kt][:, c0:c1],
                          in_=xq[kt * 128:(kt + 1) * 128, c0:c1])
            eng2 = nc.scalar if kt % 2 == 0 else nc.sync
            eng2.dma_start(out=xkvT[kt][:, c0:c1],
                           in_=xkv[kt * 128:(kt + 1) * 128, c0:c1])

    def load_rest_of_weights():
        # emitted after the first chunk's x loads so the PE isn't starved
        # at startup waiting for transposable data behind 40 weight DMAs
        w = {}
        w["k1"] = load_w(wk1, 256, "wk1", nc.sync)
        w["k2"] = load_w(wk2, 512, "wk2", nc.scalar)
        w["v1"] = load_w(wv1, 512, "wv1", nc.sync)
        w["v2"] = load_w(wv2, 256, "wv2", nc.scalar)
        w["out"] = [stat.tile([128, DQ], MM_DT, tag=f"wo{t}", name=f"wo{t}")
                    for t in range(2)]
        for t in range(2):
            nc.scalar.dma_start(out=w["out"][t][:],
                                in_=wout[t * 128:(t + 1) * 128, :])
        sink_sb = const.tile([1, HEADS], FP32, tag="sink", name="sink")
        nc.scalar.dma_start(out=sink_sb[:], in_=sink[:])
        esink = const.tile([1, HEADS], FP32, tag="esink", name="esink")
        nc.scalar.activation(esink[:], sink_sb[:], ACT.Exp)
        esink1 = const.tile([1, HEADS], FP32, tag="esink1", name="esink1")
        nc.scalar.activation(esink1[:], sink_sb[:], ACT.Exp, bias=expb[0:1, :],
                             scale=1.0)
        return w, esink, esink1

    # q/k1/k2 groups: stationary-major (one Ldweights per (w-slice, kt, half),
    # two 512-wide moving matmuls); v1+v2 fused on a shared stationary.
    def proj_groups(hf):
        groups = (
            [(qT_sb[m], wq_sb, m, xqT, None) for m in range(2)]
            + [(k1T_sb[m], wk1_sb, m, xkvT, None) for m in range(2)]
            + [(k2T_sb[m], wk2_sb, m, xkvT, None) for m in range(4)]
        )
        cols = slice(hf * 1024, (hf + 1) * 1024)
        for gi, (dst, wsb, m, xT, scale) in enumerate(groups):
            acc = ps_w.tile([128, PASS], FP32, tag="pw", name="pw")
            for kt in range(8):
                for cb in range(2):
                    c0 = hf * 1024 + cb * 512
                    mm(acc[:, cb * 512:(cb + 1) * 512],
                       wsb[kt][:, m * 128:(m + 1) * 128],
                       xT[kt][:, c0:c0 + 512],
                       start=(kt == 0), stop=(kt == 7))
            if scale is None:
                if gi % 2 == 0:
                    nc.vector.tensor_copy(dst[:, cols], acc[:])
                else:
                    nc.scalar.copy(dst[:, cols], acc[:])
            else:
                if gi % 2 == 0:
                    nc.vector.tensor_scalar_mul(dst[:, cols], acc[:], scale)
                else:
                    nc.scalar.mul(dst[:, cols], acc[:], scale)

    def proj_v(hf):
        for nb in range(8 * hf, 8 * hf + 8):
            acc = ps_w.tile([128, PASS], FP32, tag="pw", name="pw")
            for kt in range(8):
                mm(acc[:, 0:512], xkvT[kt][:, nb * 128:(nb + 1) * 128], wv1_sb[kt][:],
                   start=(kt == 0), stop=(kt == 7))
                mm(acc[:, 512:768], xkvT[kt][:, nb * 128:(nb + 1) * 128], wv2_sb[kt][:],
                   start=(kt == 0), stop=(kt == 7))
            if nb % 2 == 0:
                nc.vector.tensor_copy(v1h_sb[nb][:], acc[:, 0:512])
            else:
                nc.scalar.copy(v1h_sb[nb][:], acc[:, 0:512])
            nc.vector.tensor_sub(v1l_sb[nb][:], acc[:, 0:512], v1h_sb[nb][:])
            # pack v2 [h*64 cols] into 65-col groups with a ones column
            sv = v2a_sb[nb][:].rearrange("p (h c) -> p h c", h=HEADS)
            nc.vector.tensor_copy(
                sv[:, :, 0:64],
                acc[:, 512:768].rearrange("p (h c) -> p h c", h=HEADS))
            nc.vector.tensor_copy(
                sv[:, :, 64:65],
                ones4[:].rearrange("p (h c) -> p h c", h=HEADS))

    load_xT_half(0)
    _w, esink, esink1 = load_rest_of_weights()
    wk1_sb, wk2_sb, wv1_sb, wv2_sb, wout_sb = (
        _w["k1"], _w["k2"], _w["v1"], _w["v2"], _w["out"])
    load_xT_half(1)
    proj_groups(0)
    proj_v(0)
    proj_groups(1)
    proj_v(1)

    ps_w.release()
    wpool.release()
    xin.release()
    xt_p.release()

    # attend-phase pools (allocated after the phase-1 pools are released)
    e1p = tc.alloc_tile_pool(name="e1", bufs=1)    # 16 resident e tiles
    epool = tc.alloc_tile_pool(name="e", bufs=3)
    npool = tc.alloc_tile_pool(name="nrm", bufs=2)
    osb_p = tc.alloc_tile_pool(name="osb", bufs=2)
    ps_a = tc.alloc_tile_pool(name="ps_a", bufs=2, space="PSUM")   # 4 banks
    ps_b = tc.alloc_tile_pool(name="ps_b", bufs=1, space="PSUM")   # 2 banks
    ps_on = tc.alloc_tile_pool(name="ps_on", bufs=1, space="PSUM")  # 1 bank
    ps_bc = tc.alloc_tile_pool(name="ps_bc", bufs=1, space="PSUM")  # 1 bank
    _pools2 = [e1p, epool, npool, osb_p, ps_a, ps_b, ps_on, ps_bc]

    if PROJ_ONLY:
        for nb in range(NB):
            osb = osb_p.tile([128, DQ], FP32, tag="osb", name="osb")
            nc.vector.tensor_copy(osb[:, 0:512], v1_sb[nb][:])
            nc.vector.tensor_copy(osb[:, 512:1024], v1_sb[nb][:])
            nc.sync.dma_start(out=out[nb * 128:(nb + 1) * 128, :], in_=osb[:])
        for p_ in reversed(_pools2):
            p_.release()
        for p_ in (stat, const):
            p_.release()
        return

    # =====================================================================
    # Phase 2: attends (everything SBUF-resident)
    # =====================================================================
    def masked_exp_av(k_h, rhs_h, v_ap, out_ps, ones_ps, p):
        """One attend pass: for each key block jb, sim -> exp -> mask ->
        accumulate v.T @ e (and the ones row for attend1 denominators).

        Software-pipelined one jb deep: the PE emission order is
        sim(0), sim(1), av(0), sim(2), av(1), ... so the in-order PE queue
        never stalls on exp/mask of the block it is about to accumulate."""
        def do_sim(jb):
            simp = ps_a.tile([128, PASS], FP32, tag="sim", name="sim")
            for (a, b) in _mm_runs(jb, p):
                mm(simp[:, a:b],
                   k_h[:, jb * 128:(jb + 1) * 128],
                   rhs_h[:, a:b],
                   start=True, stop=True)
            return simp

        def do_e(jb, simp):
            runs, skip = _runs_for(jb, p)
            e = epool.tile([128, PASS], MM_DT, tag="e", name="e")
            for (t0, t1) in runs:
                nc.scalar.activation(e[:, t0 * 128:t1 * 128],
                                     simp[:, t0 * 128:t1 * 128], ACT.Exp,
                                     scale=SCALE)
            if skip is not None and jb == NB - 1:
                nc.vector.memset(e[:, skip[0] * 128:skip[1] * 128], 0.0)
            td = jb - 8 * p
            if 0 <= td < 8:   # diagonal block: keep jj <= ii
                blk = e[:, td * 128:(td + 1) * 128]
                nc.gpsimd.affine_select(
                    out=blk, in_=blk, compare_op=mybir.AluOpType.is_ge,
                    fill=0.0, base=0, pattern=[[1, 128]], channel_multiplier=-1)
            ta = jb - 4 - 8 * p
            if 0 <= ta < 8:   # jb == I+4 block: keep jj > ii
                blk = e[:, ta * 128:(ta + 1) * 128]
                nc.gpsimd.affine_select(
                    out=blk, in_=blk, compare_op=mybir.AluOpType.is_ge,
                    fill=0.0, base=-1, pattern=[[-1, 128]], channel_multiplier=1)
            return e

        def do_av(jb, e):
            segs = ([(0, 512), (512, 1024)] if jb in (0, NB - 1)
                    else _mm_runs(jb, p))
            for (a, b) in segs:
                mm(out_ps[:, a:b],
                   v_ap(jb),
                   e[:, a:b],
                   start=(jb == 0), stop=(jb == NB - 1),
                   skip_group_check=True)
            if ones_ps is not None:
                for (a, b) in segs:
                    s = a // 512
                    mm(ones_ps[32 * s:32 * s + 1, a - 512 * s:b - 512 * s],
                       onescol[:], e[:, a:b],
                       start=(jb == 0), stop=(jb == NB - 1),
                       skip_group_check=True)

        prev = None
        for jb in range(NB):
            simp = do_sim(jb)
            if prev is not None:
                do_av(jb - 1, prev)
            prev = do_e(jb, simp)
        do_av(NB - 1, prev)

    def sim_exp_1(h, p):
        """Attend1 S-stage: sims -> exp -> mask into 16 resident e tiles.
        Emitted one pass ahead so the PE has independent work during the
        previous pass's normalization chain."""
        hh = 64 * (h % 2)
        k1h = k1T_sb[h // 2][hh:hh + 64, :]
        qh = qT_sb[h // 2][hh:hh + 64, p * PASS:(p + 1) * PASS]
        es = []
        for jb in range(NB):
            simp = ps_a.tile([128, PASS], FP32, tag="sim", name="sim")
            for (a, b) in _mm_runs(jb, p):
                mm(simp[:, a:b],
                   k1h[:, jb * 128:(jb + 1) * 128],
                   qh[:, a:b],
                   start=True, stop=True)
            e = e1p.tile([128, PASS], F8, tag=f"e1_{jb}", name=f"e1_{jb}")
            runs, skip = _runs_for(jb, p)
            for (t0, t1) in runs:
                if JB_DVE[jb]:
                    nc.vector.tensor_scalar(
                        out=e[:, t0 * 128:t1 * 128].bitcast(U8),
                        in0=simp[:, t0 * 128:t1 * 128],
                        scalar1=E5_SCALE, scalar2=E5_BIAS,
                        op0=mybir.AluOpType.mult, op1=mybir.AluOpType.add)
                else:
                    nc.scalar.activation(e[:, t0 * 128:t1 * 128],
                                         simp[:, t0 * 128:t1 * 128], ACT.Exp,
                                         bias=expb[:], scale=SCALE)
            if skip is not None and jb == NB - 1:
                nc.vector.memset(e[:, skip[0] * 128:skip[1] * 128], 0.0)
            td = jb - 8 * p
            if 0 <= td < 8:
                blk = e[:, td * 128:(td + 1) * 128]
                nc.gpsimd.affine_select(
                    out=blk, in_=blk, compare_op=mybir.AluOpType.is_ge,
                    fill=0.0, base=0, pattern=[[1, 128]], channel_multiplier=-1)
            ta = jb - 4 - 8 * p
            if 0 <= ta < 8:
                blk = e[:, ta * 128:(ta + 1) * 128]
                nc.gpsimd.affine_select(
                    out=blk, in_=blk, compare_op=mybir.AluOpType.is_ge,
                    fill=0.0, base=-1, pattern=[[-1, 128]], channel_multiplier=1)
            es.append(e)
        return es

    def wout_half(p):
        """Phase 3 for the column half finished by pass group p."""
        for nb in range(8 * p, 8 * p + 8):
            pool, tag = (ps_b, "av") if nb % 2 == 0 else (ps_a, "sim")
            acc = pool.tile([128, PASS], FP32, tag=tag, name=tag)
            for s in range(2):
                for kt in range(2):
                    mm(acc[:, s * 512:(s + 1) * 512],
                       o2T[kt][:, nb * 128:(nb + 1) * 128],
                       wout_sb[kt][:, s * 512:(s + 1) * 512],
                       start=(kt == 0), stop=(kt == 1))
            osb = osb_p.tile([128, DQ], FP32, tag="osb", name="osb")
            if nb % 2 == 0:
                nc.vector.tensor_copy(osb[:], acc[:])
            else:
                nc.scalar.copy(osb[:], acc[:])
            nc.sync.dma_start(out=out[nb * 128:(nb + 1) * 128, :], in_=osb[:])

    passes = [(h, p) for p in range(2) for h in range(HEADS)]
    e1s = sim_exp_1(*passes[0])
    for idx, (h, p) in enumerate(passes):
        hh = 64 * (h % 2)

        # ------------- attend 1 V-stage: av + denominator matmuls ---------
        out1 = ps_b.tile([128, PASS], FP32, tag="av", name="av")
        ones = ps_on.tile([33, 512], FP32, tag="ones", name="ones")
        # all av matmuls first, then all denominator matmuls: onescol's
        # stationary is loaded once instead of alternating with v1 every jb
        for jb in range(NB):
            segs = ([(0, 512), (512, 1024)] if jb in (0, NB - 1)
                    else _mm_runs(jb, p))
            for (a, b) in segs:
                eab = (e1s[jb][:, a:b].bitcast(F8E5) if JB_DVE[jb]
                       else e1s[jb][:, a:b])
                for vi, vt in enumerate((v1h_sb, v1l_sb)):
                    mm(out1[:, a:b],
                       vt[jb][:, 128 * h:128 * h + 128],
                       eab,
                       start=(jb == 0 and vi == 0),
                       stop=(jb == NB - 1 and vi == 1),
                       skip_group_check=True)
        for jb in range(NB):
            segs = ([(0, 512), (512, 1024)] if jb in (0, NB - 1)
                    else _mm_runs(jb, p))
            for (a, b) in segs:
                s = a // 512
                eab = (e1s[jb][:, a:b].bitcast(F8E5) if JB_DVE[jb]
                       else e1s[jb][:, a:b])
                mm(ones[32 * s:32 * s + 1, a - 512 * s:b - 512 * s],
                   onescol[:], eab,
                   start=(jb == 0), stop=(jb == NB - 1),
                   skip_group_check=True)

        # normalize (z = out1 / denom) + silu -> hT, pipelined per
        # 512-column half: half 1's broadcast/copy overlaps half 0's DVE
        # chain, and attend2's first sim chunk can start on hT[:, 0:512]
        # while half 1 is still in flight.
        # silu(z) = z * sigmoid(z) = z / (1 + exp(-z)); stays in the
        # Exp activation table (Silu lives in a different table)
        zf = npool.tile([128, PASS], FP32, tag="z", name="z")
        rbs = npool.tile([128, PASS], FP32, tag="rb", name="rb")
        tql = npool.tile([128, PASS], FP32, tag="tq", name="tq")
        hT = npool.tile([128, PASS], F8, tag="hT", name="hT")
        for s_ in range(2):
            sl = slice(s_ * 512, (s_ + 1) * 512)
            ds_ = npool.tile([1, PASS], FP32, tag="ds", name="ds")
            nc.vector.tensor_copy(ds_[0:1, 0:512], ones[32 * s_:32 * s_ + 1, :])
            nc.vector.tensor_scalar_add(ds_[0:1, 0:512], ds_[0:1, 0:512],
                                        esink1[0:1, h:h + 1])
            nc.vector.reciprocal_approx_fast(ds_[0:1, 0:512], ds_[0:1, 0:512])
            dsb = npool.tile([1, PASS], MM_DT, tag="dsb", name="dsb")
            nc.vector.tensor_copy(dsb[0:1, 0:512], ds_[0:1, 0:512])
            rbp = ps_bc.tile([128, 512], FP32, tag="bc", name="bc")
            mm(rbp[:], onesrow[:], dsb[0:1, 0:512], start=True, stop=True)
            nc.vector.tensor_copy(rbs[:, sl], rbp[:])
            nc.vector.tensor_mul(zf[:, sl], out1[:, sl], rbs[:, sl])
            nc.scalar.activation(tql[:, sl], zf[:, sl], ACT.Exp, scale=-1.0)
            nc.vector.tensor_scalar_add(tql[:, sl], tql[:, sl], 1.0)
            nc.vector.reciprocal_approx_fast(tql[:, sl], tql[:, sl])
            nc.vector.tensor_mul(hT[:, sl], zf[:, sl], tql[:, sl])

        # next pass's S-stage: fills the PE while the chain above runs
        if idx + 1 < len(passes):
            e1s = sim_exp_1(*passes[idx + 1])

        # ------------- attend 2 (fused jb-pipelined) -------------
        k2h = k2T_sb[h][:]
        out2 = ps_b.tile([65, PASS], FP32, tag="av", name="av")
        masked_exp_av(
            k2h, hT[:], lambda jb: v2a_sb[jb][:, 65 * h:65 * h + 65],
            out2[:], None, p)

        # normalize attend2 (denominator rode along as row 64)
        d2 = npool.tile([1, PASS], FP32, tag="ds", name="ds")
        nc.vector.tensor_copy(d2[:], out2[64:65, :])
        nc.vector.tensor_scalar_add(d2[:], d2[:], esink[0:1, h:h + 1])
        nc.vector.reciprocal_approx_fast(d2[:], d2[:])
        d2b = npool.tile([1, PASS], MM_DT, tag="dsb", name="dsb")
        nc.vector.tensor_copy(d2b[:], d2[:])
        rbs2 = npool.tile([64, PASS], FP32, tag="rb2", name="rb2")
        for s_ in range(2):
            rbp = ps_bc.tile([128, 512], FP32, tag="bc", name="bc")
            mm(rbp[0:64, :], onesrow[0:1, 0:64],
               d2b[0:1, s_ * 512:(s_ + 1) * 512], start=True, stop=True)
            nc.vector.tensor_copy(rbs2[:, s_ * 512:(s_ + 1) * 512], rbp[0:64, :])
        dst = o2T[h // 2][hh:hh + 64, p * PASS:(p + 1) * PASS]
        nc.vector.tensor_mul(dst, out2[0:64, :], rbs2[:])

        if DEBUG and h == 0 and p == 0:
            nc.sync.dma_start(out=io["dbg_hT"].bitcast(MM_DT), in_=hT[:])
            dzf = npool.tile([128, PASS], FP32, tag="dzf", name="dzf")
            nc.vector.tensor_copy(dzf[:], zf[:])
            nc.sync.dma_start(out=io["dbg_zf"], in_=dzf[:])
            do2 = npool.tile([65, PASS], FP32, tag="do2", name="do2")
            nc.vector.tensor_copy(do2[:], out2[:])
            nc.sync.dma_start(out=io["dbg_out2"], in_=do2[:])

        # interleave the output projection for the completed column half
        if idx == len(passes) - 1 or (idx + 1 < len(passes)
                                      and passes[idx + 1][1] != p):
            wout_half(p)

    if DEBUG:
        for t in range(2):
            nc.sync.dma_start(out=io["dbg_qT"][t * 128:(t + 1) * 128, :].bitcast(MM_DT),
                              in_=qT_sb[t][:])
            nc.sync.dma_start(out=io["dbg_k1T"][t * 128:(t + 1) * 128, :].bitcast(MM_DT),
                              in_=k1T_sb[t][:])
            nc.sync.dma_start(out=io["dbg_o2T"][t * 128:(t + 1) * 128, :].bitcast(MM_DT),
                              in_=o2T[t][:])
        for t in range(4):
            nc.sync.dma_start(out=io["dbg_v1"][t * 128:(t + 1) * 128, :].bitcast(MM_DT),
                              in_=v1_sb[t][:])

    for p_ in reversed(_pools2):
        p_.release()
    for p_ in (stat, const):
        p_.release()


_NC_CACHE = {}


def build_nc():
    key = (str(MM_DT), REPS, DEBUG, PROJ_ONLY)
    if key in _NC_CACHE:
        return _NC_CACHE[key]
    nc = bacc.Bacc("TRN2", target_bir_lowering=False, debug=False,
                   num_devices=N_CORES)
    io = {
        "xq": nc.dram_tensor("xq", [DQ, N], MM_DT, kind="ExternalInput").ap(),
        "xkv": nc.dram_tensor("xkv", [DQ, N], MM_DT, kind="ExternalInput").ap(),
        "wq": nc.dram_tensor("wq", [DQ, 256], MM_DT, kind="ExternalInput").ap(),
        "wk1": nc.dram_tensor("wk1", [DQ, 256], MM_DT, kind="ExternalInput").ap(),
        "wv1": nc.dram_tensor("wv1", [DQ, 512], MM_DT, kind="ExternalInput").ap(),
        "wk2": nc.dram_tensor("wk2", [DQ, 512], MM_DT, kind="ExternalInput").ap(),
        "wv2": nc.dram_tensor("wv2", [DQ, 256], MM_DT, kind="ExternalInput").ap(),
        "wout": nc.dram_tensor("wout", [256, DQ], MM_DT, kind="ExternalInput").ap(),
        "sink": nc.dram_tensor("sink", [1, HEADS], FP32, kind="ExternalInput").ap(),
        "out": nc.dram_tensor("out", [N, DQ], FP32, kind="ExternalOutput").ap(),
    }
    if DEBUG:
        for nm, shp, dt in (("dbg_qT", [256, N], FP32), ("dbg_k1T", [256, N], FP32),
                            ("dbg_o2T", [256, N], FP32), ("dbg_v1", [512, 512], FP32),
                            ("dbg_hT", [128, PASS], FP32), ("dbg_zf", [128, PASS], FP32),
                            ("dbg_out2", [65, PASS], FP32)):
            shp2 = list(shp)
            if dt is FP32 and nm in ("dbg_qT", "dbg_k1T", "dbg_o2T", "dbg_v1", "dbg_hT"):
                shp2[-1] = shp[-1] // 2   # bf16 payload bitcast into fp32 words
            io[nm] = nc.dram_tensor(nm, shp2, FP32, kind="ExternalOutput").ap()
    if REPS == 0:
        # extra input so the I/O-only program's jax trace-cache key differs
        # from the real kernel's (the cache ignores the BIR payload)
        io["dummy0"] = nc.dram_tensor("dummy0", [1, 8], FP32,
                                      kind="ExternalInput").ap()
    with TileContext(nc) as tc:
        if REPS == 0:
            pool0 = tc.alloc_tile_pool(name="p0", bufs=1)
            t0_ = pool0.tile([128, N], MM_DT, name="t0_")
            nc.sync.dma_start(out=t0_[:], in_=io["xq"][0:128, :])
            o0_ = pool0.tile([128, DQ], FP32, name="o0_")
            nc.vector.tensor_copy(o0_[:], t0_[:, 0:DQ])
            for nb in range(NB):
                nc.sync.dma_start(out=io["out"][nb * 128:(nb + 1) * 128, :],
                                  in_=o0_[:])
            pool0.release()
        for _ in range(REPS):
            build_kernel(nc, tc, io)
    nc.compile()
    _NC_CACHE[key] = (nc, io)
    return nc, io


_BF16 = None


def _bf16():
    global _BF16
    if _BF16 is None:
        import ml_dtypes
        _BF16 = np.dtype(ml_dtypes.bfloat16)
    return _BF16


def make_in_maps(inputs):
    bf = _bf16()
    xq_b = [np.ascontiguousarray(np.asarray(inputs["queries_input"][b]).T)
            .astype(bf) for b in range(2)]
    xkv_b = [np.ascontiguousarray(np.asarray(inputs["key_values_input"][b]).T)
             .astype(bf) for b in range(2)]
    in_maps = []
    for c in range(N_CORES):
        b, g = c // 4, c % 4
        s64 = slice(g * 256, (g + 1) * 256)
        s128 = slice(g * 512, (g + 1) * 512)
        in_maps.append({
            "xq": xq_b[b],
            "xkv": xkv_b[b],
            "wq": np.ascontiguousarray(inputs["Wq"][:, s64]).astype(bf),
            "wk1": np.ascontiguousarray(inputs["Wk1"][:, s64]).astype(bf),
            "wv1": np.ascontiguousarray(inputs["Wv1"][:, s128]).astype(bf),
            "wk2": np.ascontiguousarray(inputs["Wk2"][:, s128]).astype(bf),
            "wv2": np.ascontiguousarray(inputs["Wv2"][:, s64]).astype(bf),
            "wout": np.ascontiguousarray(inputs["Wout"][s64, :]).astype(bf),
            "sink": np.ascontiguousarray(
                inputs["attn_sink"][g * 4:(g + 1) * 4]).reshape(1, HEADS)
                .astype(np.float32),
        })
    return in_maps


def kernel(**inputs):
    from concourse.bass_utils import run_bass_kernel_spmd

    inputs = {k: np.asarray(v) for k, v in inputs.items()}
    nc, _ = build_nc()
    in_maps = make_in_maps(inputs)
    res = run_bass_kernel_spmd(nc, in_maps, list(range(N_CORES)))
    out = np.zeros((2, N, DQ), dtype=np.float32)
    for c in range(N_CORES):
        out[c // 4] += res.results[c]["out"]
    return out

